# revision 1
# baseline (speedup 1.0000x reference)
"""BitSwiGLU Trainium2 kernel (8 NeuronCores, data-parallel over tokens).

Math (per bit_linear, forward values):
    gamma_x = clip(max|x_row|, 1e-5);  k = rne(x * 127/gamma_x)  in [-127,127]
    gamma_w = clip(mean|w|, 1e-5);    t = sign(w) * (|w| > 0.5*gamma_w)  in {-1,0,1}
    y = (k @ t.T) * (gamma_x*gamma_w/127) + b

k and t are small integers, exactly representable in bf16; the TensorEngine
accumulates bf16 products in fp32 PSUM, so k @ t.T is EXACT integer math at
bf16 speed. All scales are applied per-token (per-partition) at PSUM eviction.

Ternarization runs as t2 = sign(w - thr) + sign(w + thr) in {-2,0,2}
(two ScalarE Sign ops + one bf16 VectorE add; fp32 subtract-sign is exact,
so the comparison against thr = 0.5*gamma is bit-exact). The factor 2 is
folded into the eviction scales (exact power of two).

Sharding: data-parallel -- 8192 tokens split 1024/core; weights replicated.
Each core ternarizes weights locally, writes them to DRAM as bf16 in
natural layout, and the matmul phases transpose-load [K,512] tiles through
the DMA XBAR.

silu(y) is computed as y * sigmoid(y) (Sigmoid on ScalarE).
Biases are zero in this problem; gate/val biases are asserted zero host-side
and out_b is added on host.
"""

import numpy as np

import concourse.bass as bass
import concourse.mybir as mybir
import concourse.tile as tile
from concourse import bacc
from concourse import bass_isa
from concourse.bass_utils import run_bass_kernel_spmd

F32 = mybir.dt.float32
BF16 = mybir.dt.bfloat16
AF = mybir.ActivationFunctionType
OP = mybir.AluOpType
AX = mybir.AxisListType

MAGIC = 12582912.0  # 1.5 * 2**23 : (v + MAGIC) - MAGIC == rne(v) for |v| < 2**22

N_CORES = 8


def _build(T, D, H, n_cores=N_CORES):
    """Build + compile the per-core Bass program. All cores run the same
    program on their own token shard (weights replicated)."""
    nc = bacc.Bacc("TRN2", target_bir_lowering=False, debug=False,
                   num_devices=n_cores)
    x_d = nc.dram_tensor("x", [T, D], F32, kind="ExternalInput")
    gw_d = nc.dram_tensor("gate_w", [H, D], F32, kind="ExternalInput")
    vw_d = nc.dram_tensor("val_w", [H, D], F32, kind="ExternalInput")
    ow_d = nc.dram_tensor("out_w", [D, H], F32, kind="ExternalInput")
    out_d = nc.dram_tensor("out", [T, D], F32, kind="ExternalOutput")

    with tile.TileContext(nc) as tc:
        _body(tc, x_d, gw_d, vw_d, ow_d, out_d, T=T, D=D, H=H)
    nc.compile()
    return nc


def _body(tc, x_d, gw_d, vw_d, ow_d, out_d, *, T, D, H):
    nc = tc.nc
    KD = D // 128      # contraction chunks, mm1
    KH = H // 128      # contraction chunks, mm2
    NH = H // 512      # hidden 512-chunks (mm1 output tiles)
    ND = D // 512      # d_out 512-chunks (mm2 output tiles)
    MT = T // 128      # token chunks
    RG = H // 128      # gate/val weight row-chunks
    RO = D // 128      # out_w row-chunks
    CW = min(2048, D)  # gate/val weight processing width
    NW = D // CW
    CO = min(2048, H)  # out_w weight processing width
    NO = H // CO
    CQ = min(2048, H)  # h-quant processing chunk
    NQ = H // CQ
    MHALF = max(1, MT // 2)

    Xv = x_d.ap().rearrange("(m p) d -> m p d", p=128)
    Ov = out_d.ap().rearrange("(m p) d -> m p d", p=128)

    with (
        tc.tile_pool(name="persist", bufs=1) as pp,
        tc.tile_pool(name="psp", bufs=8, space="PSUM") as psp,
        tc.tile_pool(name="drp", bufs=1, space="DRAM") as drp,
    ):
        # DRAM scratch: ternary {-1,0,1}*2 weights (bf16, natural layout;
        # gate/val split per 512-row slice so mm1 reads pipeline with the
        # ternarize writes) + h
        gq_l = [drp.tile([512, D], BF16, tag=f"gq{n}", name=f"gq{n}")
                for n in range(NH)]
        vq_l = [drp.tile([512, D], BF16, tag=f"vq{n}", name=f"vq{n}")
                for n in range(NH)]
        oq_d = drp.tile([D, H], BF16, tag="oq")
        h_d = drp.tile([MT, 128, H], F32, tag="h")

        s1, s12, gx_l, hmax = [], [], [], []
        for m in range(MT):
            for nm, lst in (("s1", s1), ("s12", s12), ("gx", gx_l),
                            ("hmax", hmax)):
                t = pp.tile([128, 1], F32, tag=f"{nm}{m}", name=f"{nm}{m}")
                lst.append(t)
        hp = [pp.tile([128, NH], F32, tag=f"hp{m}", name=f"hp{m}")
              for m in range(MT)]

        with tc.tile_pool(name="kxp", bufs=1) as kxp:
            # ---------------- x quantization + transpose ----------------
            # kxT[p=d, k, t] = k_x[t, k*128+p]
            kxT = kxp.tile([128, KD, T], BF16, tag="kxT")
            with tc.tile_pool(name="xst", bufs=3) as xst:
                for m in range(MT):
                    xt = xst.tile([128, D], F32, tag="x_in")
                    nc.sync.dma_start(out=xt[:, :], in_=Xv[m])
                    gx = gx_l[m]
                    nc.vector.tensor_reduce(out=gx[:, :], in_=xt[:, :],
                                            axis=AX.X, op=OP.max,
                                            apply_absolute_value=True)
                    nc.vector.tensor_scalar_max(out=gx[:, :], in0=gx[:, :],
                                                scalar1=1e-5)
                    rcp = xst.tile([128, 1], F32, tag="rcpx")
                    nc.vector.reciprocal(out=rcp[:, :], in_=gx[:, :])
                    sx = xst.tile([128, 1], F32, tag="sx")
                    nc.vector.tensor_scalar_mul(out=sx[:, :], in0=rcp[:, :],
                                                scalar1=127.0)
                    # k_x = rne(x * sx) -> bf16 (exact small ints)
                    xs = xst.tile([128, D], F32, tag="x_sc")
                    nc.scalar.activation(out=xs[:, :], in_=xt[:, :],
                                         func=AF.Copy, scale=sx[:, :])
                    kx = xst.tile([128, D], BF16, tag="kx")
                    nc.vector.tensor_scalar(out=kx[:, :], in0=xs[:, :],
                                            scalar1=MAGIC, scalar2=MAGIC,
                                            op0=OP.add, op1=OP.subtract)
                    nc.sync.dma_start(out=kxT[:, :, m * 128:(m + 1) * 128],
                                      in_=kx[:, :], transpose=True)

            # ---------------- weight prep ----------------
            with tc.tile_pool(name="wp", bufs=3) as wp:
                # gamma = clip(mean|w|, 1e-5); thr = 0.5*gamma
                def gamma_of(w_ap, R, C, NC_, label):
                    CWc = C // NC_
                    Wv = w_ap.rearrange("(r p) c -> r p c", p=128)
                    parts = pp.tile([128, R * NC_], F32, tag=f"parts_{label}",
                                    name=f"parts_{label}")
                    for r in range(R):
                        for j in range(NC_):
                            wt = wp.tile([128, CWc], F32, tag="g_in")
                            nc.sync.dma_start(
                                out=wt[:, :],
                                in_=Wv[r][:, j * CWc:(j + 1) * CWc])
                            scr = wp.tile([128, CWc], F32, tag="g_scr")
                            nc.scalar.activation(
                                out=scr[:, :], in_=wt[:, :], func=AF.Abs,
                                accum_out=parts[:,
                                                r * NC_ + j:r * NC_ + j + 1])
                    tot = pp.tile([128, 1], F32, tag=f"gsum_{label}",
                                  name=f"gsum_{label}")
                    nc.vector.tensor_reduce(out=tot[:, :], in_=parts[:, :],
                                            axis=AX.X, op=OP.add)
                    nc.gpsimd.partition_all_reduce(tot[:, :], tot[:, :], 128,
                                                   bass_isa.ReduceOp.add)
                    g = pp.tile([128, 1], F32, tag=f"gamma_{label}",
                                name=f"gamma_{label}")
                    nc.vector.tensor_scalar(out=g[:, :], in0=tot[:, :],
                                            scalar1=1.0 / (R * 128 * C),
                                            scalar2=1e-5, op0=OP.mult,
                                            op1=OP.max)
                    thr = pp.tile([128, 1], F32, tag=f"thr_{label}",
                                  name=f"thr_{label}")
                    nc.vector.tensor_scalar_mul(out=thr[:, :], in0=g[:, :],
                                                scalar1=0.5)
                    nthr = pp.tile([128, 1], F32, tag=f"nthr_{label}",
                                   name=f"nthr_{label}")
                    nc.vector.tensor_scalar_mul(out=nthr[:, :], in0=thr[:, :],
                                                scalar1=-1.0)
                    return g, thr, nthr

                g_gw, thr_g, nthr_g = gamma_of(gw_d.ap(), RG, D, NW, "g")
                g_vw, thr_v, nthr_v = gamma_of(vw_d.ap(), RG, D, NW, "v")
                g_ow, thr_o, nthr_o = gamma_of(ow_d.ap(), RO, H, NO, "o")

                # per-token eviction scales; /254 folds the ternary 2x
                for m in range(MT):
                    nc.vector.tensor_scalar(out=s1[m][:, :],
                                            in0=gx_l[m][:, :],
                                            scalar1=g_gw[:, :],
                                            scalar2=1.0 / 254.0,
                                            op0=OP.mult, op1=OP.mult)
                    s2 = wp.tile([128, 1], F32, tag="s2tmp")
                    nc.vector.tensor_scalar(out=s2[:, :], in0=gx_l[m][:, :],
                                            scalar1=g_vw[:, :],
                                            scalar2=1.0 / 254.0,
                                            op0=OP.mult, op1=OP.mult)
                    nc.vector.tensor_mul(out=s12[m][:, :], in0=s1[m][:, :],
                                         in1=s2[:, :])

                # ternarize: t2 = sign(w-thr) + sign(w+thr) in {-2,0,2};
                # dve=True uses 2*is_gt(w,thr) - 2*is_lt(w,-thr) on VectorE
                # (same values, spreads the load off ScalarE)
                def quant_row(Wv, dst_ap, r, j, CWc, thr, nthr, dve=False):
                    sl = slice(j * CWc, (j + 1) * CWc)
                    wt = wp.tile([128, CWc], F32, tag="q_in")
                    nc.sync.dma_start(out=wt[:, :], in_=Wv[r][:, sl])
                    tq = wp.tile([128, CWc], BF16, tag="q_tq")
                    if dve:
                        mp = wp.tile([128, CWc], BF16, tag="q_mp")
                        nc.vector.tensor_scalar(out=mp[:, :], in0=wt[:, :],
                                                scalar1=thr[:, :],
                                                scalar2=2.0,
                                                op0=OP.is_gt, op1=OP.mult)
                        mn = wp.tile([128, CWc], BF16, tag="q_mn")
                        nc.vector.tensor_scalar(out=mn[:, :], in0=wt[:, :],
                                                scalar1=nthr[:, :],
                                                scalar2=2.0,
                                                op0=OP.is_lt, op1=OP.mult)
                        nc.vector.tensor_sub(out=tq[:, :], in0=mp[:, :],
                                             in1=mn[:, :])
                    else:
                        sp = wp.tile([128, CWc], BF16, tag="q_sp")
                        nc.scalar.activation(out=sp[:, :], in_=wt[:, :],
                                             func=AF.Sign, bias=nthr[:, :])
                        sn = wp.tile([128, CWc], BF16, tag="q_sn")
                        nc.scalar.activation(out=sn[:, :], in_=wt[:, :],
                                             func=AF.Sign, bias=thr[:, :])
                        nc.vector.tensor_add(out=tq[:, :], in0=sp[:, :],
                                             in1=sn[:, :])
                    nc.sync.dma_start(out=dst_ap[:, sl], in_=tq[:, :])

                Gv = gw_d.ap().rearrange("(r p) c -> r p c", p=128)
                Vv = vw_d.ap().rearrange("(r p) c -> r p c", p=128)
                for r in range(RG):
                    rr = r % 4
                    for j in range(NW):
                        quant_row(Gv, gq_l[r // 4][rr * 128:(rr + 1) * 128],
                                  r, j, CW, thr_g, nthr_g)
                        quant_row(Vv, vq_l[r // 4][rr * 128:(rr + 1) * 128],
                                  r, j, CW, thr_v, nthr_v, dve=True)
                Owv = ow_d.ap().rearrange("(r p) c -> r p c", p=128)
                Oq = oq_d[:, :].rearrange("(r p) c -> r p c", p=128)
                for r in range(RO):
                    for j in range(NO):
                        quant_row(Owv, Oq[r], r, j, CO, thr_o, nthr_o)

            # ---------------- mm1: gate/val matmuls + h ----------------
            with tc.tile_pool(name="m1p", bufs=2) as m1p:
                for n in range(NH):
                    # transpose-load weight slices [128=d(k), 512=h(n)]
                    wg_n = m1p.tile([128, KD, 512], BF16, tag="wg_n")
                    wv_n = m1p.tile([128, KD, 512], BF16, tag="wv_n")
                    for k in range(KD):
                        nc.sync.dma_start(
                            out=wg_n[:, k, :],
                            in_=gq_l[n][:, k * 128:(k + 1) * 128],
                            transpose=True)
                        nc.sync.dma_start(
                            out=wv_n[:, k, :],
                            in_=vq_l[n][:, k * 128:(k + 1) * 128],
                            transpose=True)
                    for half in range(MT // MHALF):
                        ms = range(half * MHALF, (half + 1) * MHALF)
                        pg = {m: psp.tile([128, 512], F32, tag="ps",
                                          name=f"pg{n}_{m}") for m in ms}
                        pv = {m: psp.tile([128, 512], F32, tag="ps",
                                          name=f"pv{n}_{m}") for m in ms}
                        for k in range(KD):
                            for m in ms:
                                lhsT = kxT[:, k, m * 128:(m + 1) * 128]
                                nc.tensor.matmul(pg[m][:, :], lhsT=lhsT,
                                                 rhs=wg_n[:, k, :],
                                                 start=(k == 0),
                                                 stop=(k == KD - 1))
                                nc.tensor.matmul(pv[m][:, :], lhsT=lhsT,
                                                 rhs=wv_n[:, k, :],
                                                 start=(k == 0),
                                                 stop=(k == KD - 1))
                        for m in ms:
                            A = m1p.tile([128, 512], F32, tag="Asb",
                                         bufs=MHALF + 2, name=f"A{n}_{m}")
                            nc.scalar.activation(out=A[:, :], in_=pg[m][:, :],
                                                 func=AF.Sigmoid,
                                                 scale=s1[m][:, :])
                            B = m1p.tile([128, 512], F32, tag="Bsb",
                                         bufs=MHALF + 2, name=f"B{n}_{m}")
                            nc.scalar.activation(out=B[:, :], in_=pg[m][:, :],
                                                 func=AF.Copy,
                                                 scale=s12[m][:, :])
                            tmp = m1p.tile([128, 512], F32, tag="tmp", bufs=4,
                                           name=f"tmp{n}_{m}")
                            nc.vector.tensor_mul(out=tmp[:, :],
                                                 in0=pv[m][:, :],
                                                 in1=B[:, :])
                            hs = m1p.tile([128, 512], F32, tag="hsl", bufs=4,
                                          name=f"hs{n}_{m}")
                            nc.vector.tensor_mul(out=hs[:, :], in0=A[:, :],
                                                 in1=tmp[:, :])
                            nc.vector.tensor_reduce(
                                out=hp[m][:, n:n + 1], in_=hs[:, :],
                                axis=AX.X, op=OP.max,
                                apply_absolute_value=True)
                            nc.sync.dma_start(
                                out=h_d[m, :, n * 512:(n + 1) * 512],
                                in_=hs[:, :])

        # ---------------- h quantization + mm2 ----------------
        with tc.tile_pool(name="khp", bufs=1) as khp:
            khT, s_out = [], []
            with tc.tile_pool(name="hqp", bufs=3) as hqp:
                for m in range(MT):
                    nc.vector.tensor_reduce(out=hmax[m][:, :],
                                            in_=hp[m][:, :], axis=AX.X,
                                            op=OP.max)
                    gh = hqp.tile([128, 1], F32, tag="gh")
                    nc.vector.tensor_scalar_max(out=gh[:, :],
                                                in0=hmax[m][:, :],
                                                scalar1=1e-5)
                    rch = hqp.tile([128, 1], F32, tag="rch")
                    nc.vector.reciprocal(out=rch[:, :], in_=gh[:, :])
                    sh = hqp.tile([128, 1], F32, tag="sh")
                    nc.vector.tensor_scalar_mul(out=sh[:, :], in0=rch[:, :],
                                                scalar1=127.0)
                    so = pp.tile([128, 1], F32, tag=f"so{m}", name=f"so{m}")
                    nc.vector.tensor_scalar(out=so[:, :], in0=gh[:, :],
                                            scalar1=g_ow[:, :],
                                            scalar2=1.0 / 254.0,
                                            op0=OP.mult, op1=OP.mult)
                    s_out.append(so)
                    kT = khp.tile([128, KH, 128], BF16, tag=f"khT{m}",
                                  name=f"khT{m}")
                    khT.append(kT)
                    for q in range(NQ):
                        hc = hqp.tile([128, CQ], F32, tag="h_rd")
                        nc.sync.dma_start(out=hc[:, :],
                                          in_=h_d[m, :, q * CQ:(q + 1) * CQ])
                        hsc = hqp.tile([128, CQ], F32, tag="h_sc")
                        nc.scalar.activation(out=hsc[:, :], in_=hc[:, :],
                                             func=AF.Copy, scale=sh[:, :])
                        kh = hqp.tile([128, CQ], BF16, tag="kh")
                        nc.vector.tensor_scalar(out=kh[:, :], in0=hsc[:, :],
                                                scalar1=MAGIC, scalar2=MAGIC,
                                                op0=OP.add, op1=OP.subtract)
                        nc.sync.dma_start(
                            out=kT[:, q * (CQ // 128):(q + 1) * (CQ // 128),
                                   :],
                            in_=kh[:, :], transpose=True)

            with tc.tile_pool(name="m2p", bufs=3) as m2p:
                for c in range(ND):
                    po = [psp.tile([128, 512], F32, tag="ps",
                                   name=f"po{c}_{m}") for m in range(MT)]
                    for k in range(KH):
                        wo = m2p.tile([128, 512], BF16, tag="wo", bufs=4)
                        nc.sync.dma_start(
                            out=wo[:, :],
                            in_=oq_d[c * 512:(c + 1) * 512,
                                     k * 128:(k + 1) * 128],
                            transpose=True)
                        for m in range(MT):
                            nc.tensor.matmul(po[m][:, :],
                                             lhsT=khT[m][:, k, :],
                                             rhs=wo[:, :],
                                             start=(k == 0),
                                             stop=(k == KH - 1))
                    for m in range(MT):
                        ot = m2p.tile([128, 512], F32, tag="ot", bufs=4,
                                      name=f"ot{c}_{m}")
                        nc.scalar.activation(out=ot[:, :], in_=po[m][:, :],
                                             func=AF.Copy,
                                             scale=s_out[m][:, :])
                        nc.sync.dma_start(
                            out=Ov[m][:, c * 512:(c + 1) * 512],
                            in_=ot[:, :])


_NC_CACHE = {}


def _get_nc(T, D, H):
    key = (T, D, H)
    if key not in _NC_CACHE:
        _NC_CACHE[key] = _build(T, D, H)
    return _NC_CACHE[key]


def kernel(x, gate_w, gate_b, val_w, val_b, out_w, out_b, _trace=False):
    x = np.ascontiguousarray(np.asarray(x), dtype=np.float32)
    gate_w = np.ascontiguousarray(np.asarray(gate_w), dtype=np.float32)
    val_w = np.ascontiguousarray(np.asarray(val_w), dtype=np.float32)
    out_w = np.ascontiguousarray(np.asarray(out_w), dtype=np.float32)
    gate_b = np.asarray(gate_b)
    val_b = np.asarray(val_b)
    out_b = np.asarray(out_b)
    assert not np.any(gate_b) and not np.any(val_b), (
        "device kernel folds silu(y+b) with b=0; nonzero gate/val bias "
        "not supported")

    orig_shape = x.shape
    xf = x.reshape(-1, x.shape[-1])
    n_tok, d = xf.shape
    h = gate_w.shape[0]
    t_core = n_tok // N_CORES

    nc = _get_nc(t_core, d, h)
    in_maps = [
        {
            "x": xf[i * t_core:(i + 1) * t_core],
            "gate_w": gate_w,
            "val_w": val_w,
            "out_w": out_w,
        }
        for i in range(N_CORES)
    ]
    res = run_bass_kernel_spmd(nc, in_maps, core_ids=list(range(N_CORES)),
                               trace=_trace)
    out = np.concatenate([res.results[i]["out"] for i in range(N_CORES)],
                         axis=0)
    out = out + out_b[None, :].astype(np.float32)
    kernel._last_results = res
    return out.reshape(orig_shape)



# revision 5
# speedup vs baseline: 1.4567x; 1.4567x over previous
"""BitSwiGLU Trainium2 kernel — tensor-parallel over hidden, 8 NeuronCores.

Math (per bit_linear, forward values):
    gamma_x = clip(max|x_row|, 1e-5);  k = rne(x * 127/gamma_x)  in [-127,127]
    gamma_w = clip(mean|w|, 1e-5);    t = sign(w) * (|w| > 0.5*gamma_w)
    y = (k @ t.T) * (gamma_x*gamma_w/127) + b

k and t are small integers, exactly representable in bf16; the TensorEngine
accumulates bf16 products in fp32 PSUM, so k @ t.T is EXACT integer math at
bf16 speed. Ternarization runs as t2 = sign(w-thr) + sign(w+thr) in {-2,0,2};
the factor 2 is folded into the eviction scales.

Sharding (vs. the data-parallel baseline, which re-read all 200MB of f32
weights on every core): hidden is sharded 8 ways.
  - Each core holds 1/8 of gate/val/out weights (host passes them
    pre-transposed to the matmul-friendly [contract, free] layout), so
    per-core weight traffic drops 8x.
  - x is quantized data-parallel (each core does its 1024 tokens), then the
    transposed bf16 integer levels kxT are AllGathered.
  - Global gamma_w for each weight = tiny AllReduce of per-core |w| sums.
  - mm1: each core computes h[:, its 1024 hidden] for ALL 8192 tokens.
    Per 1024-token block r: per-token max|h| over the local hidden slice is
    AllReduce(max)-ed (4KB), h is re-quantized to bf16 integer levels with
    the exact global gamma_h, and an AllToAll gives core j the fully
    assembled quantized h rows for its 128-token sub-chunk of block r.
  - mm2 is then fully local (token-parallel) against the AllGathered bf16
    ternary out_w — no reduction collective on the tail, output is exact
    fp32.
  Collectives run on TOPSP+SDMA silicon and overlap the matmuls.

Token ownership: core i owns global tokens {r*1024 + i*128 + [0,128) for
r in 0..7}; the host wrapper re-interleaves the 8 per-core outputs.
"""

import numpy as np

import concourse.mybir as mybir
import concourse.tile as tile
from concourse import bacc
from concourse import bass_isa
from concourse.bass_utils import run_bass_kernel_spmd

F32 = mybir.dt.float32
BF16 = mybir.dt.bfloat16
AF = mybir.ActivationFunctionType
OP = mybir.AluOpType
AX = mybir.AxisListType

MAGIC = 12582912.0  # 1.5 * 2**23 : (v + MAGIC) - MAGIC == rne(v) for |v| < 2**22

N_CORES = 8
D = 2048            # d_model
H = 8192            # hidden (full)
HL = H // N_CORES   # 1024 hidden per core
T = 8192            # total tokens
TL = T // N_CORES   # 1024 tokens quantized per core
KD = D // 128       # 16 contraction chunks, mm1
KHL = HL // 128     # 8  chunks of the local hidden slice
KH = H // 128       # 64 contraction chunks, mm2
RG = [list(range(N_CORES))]


def _build():
    nc = bacc.Bacc("TRN2", target_bir_lowering=False, debug=False,
                   num_devices=N_CORES)
    x_d = nc.dram_tensor("x", [TL, D], F32, kind="ExternalInput")
    gwT_d = nc.dram_tensor("gwT", [D, HL], F32, kind="ExternalInput")
    vwT_d = nc.dram_tensor("vwT", [D, HL], F32, kind="ExternalInput")
    owT_d = nc.dram_tensor("owT", [HL, D], F32, kind="ExternalInput")
    sel_d = nc.dram_tensor("sel8", [1, N_CORES], F32, kind="ExternalInput")
    out_d = nc.dram_tensor("out", [TL, D], F32, kind="ExternalOutput")

    with tile.TileContext(nc) as tc:
        _body(tc, x_d, gwT_d, vwT_d, owT_d, sel_d, out_d)
    nc.compile()
    return nc


def _body(tc, x_d, gwT_d, vwT_d, owT_d, sel_d, out_d):
    nc = tc.nc
    gp = nc.gpsimd

    with (
        tc.tile_pool(name="pp", bufs=1) as pp,
        tc.tile_pool(name="psp", bufs=8, space="PSUM") as psp,
        tc.tile_pool(name="drp", bufs=1, space="DRAM") as drp,
    ):
        # ---------- DRAM scratch ----------
        kxb = drp.tile([KD, 128, TL], BF16, tag="kxb")          # AG in
        kxg = drp.tile([N_CORES, KD, 128, TL], BF16, tag="kxg",
                       addr_space="Shared")                      # AG out
        gxb = drp.tile([1, TL], F32, tag="gxb")
        gxg = drp.tile([N_CORES, TL], F32, tag="gxg", addr_space="Shared")
        grb_i = drp.tile([1, 4], F32, tag="grb_i")
        grb_o = drp.tile([1, 4], F32, tag="grb_o", addr_space="Shared")
        w2b = drp.tile([HL, D], BF16, tag="w2b")                # AG in
        w2g = drp.tile([N_CORES * HL, D], BF16, tag="w2g",
                       addr_space="Shared")                      # AG out
        hsp = [drp.tile([KHL, 128, HL], F32, tag=f"hsp{r}", name=f"hsp{r}")
               for r in range(N_CORES)]
        arh_i = [drp.tile([1, TL], F32, tag=f"arhi{r}", name=f"arhi{r}")
                 for r in range(N_CORES)]
        arh_o = [drp.tile([1, TL], F32, tag=f"arho{r}", name=f"arho{r}",
                          addr_space="Shared") for r in range(N_CORES)]
        a2i = [drp.tile([N_CORES, 128, HL], BF16, tag=f"a2i{r}",
                        name=f"a2i{r}") for r in range(N_CORES)]
        a2o = [drp.tile([N_CORES, 128, HL], BF16, tag=f"a2o{r}",
                        name=f"a2o{r}") for r in range(N_CORES)]

        # ---------- persistent SBUF (whole program) ----------
        gam = pp.tile([128, 4], F32, tag="gam")             # g, v, o gammas
        thr = pp.tile([128, 6], F32, tag="thr")             # +-thr g/v/o
        s1a = pp.tile([128, 64], F32, tag="s1a")
        s12a = pp.tile([128, 64], F32, tag="s12a")
        selb = pp.tile([128, N_CORES], F32, tag="selb")
        sofull = [pp.tile([128, KHL], F32, tag=f"sofull{r}",
                          name=f"sofull{r}") for r in range(N_CORES)]
        sosel = [pp.tile([128, 1], F32, tag=f"sosel{r}", name=f"sosel{r}")
                 for r in range(N_CORES)]
        # mm2 lhsT, k-half 1 (hidden sources j=0..3): assembled during mm1
        khT1 = pp.tile([128, KH // 2, TL], BF16, tag="khT1")    # 8.4 MB

        # ================= phase X: x quantization =================
        Xv = x_d.ap().rearrange("(m p) d -> m p d", p=128)
        with tc.tile_pool(name="xqo", bufs=1) as xqo:
            kxT = xqo.tile([128, KD, TL], BF16, tag="kxT")      # 4.2 MB
            gxall = xqo.tile([128, TL // 128], F32, tag="gxall")
            with tc.tile_pool(name="xq", bufs=3) as xq:
                for m in range(TL // 128):
                    xt = xq.tile([128, D], F32, tag="x_in")
                    nc.sync.dma_start(out=xt[:, :], in_=Xv[m])
                    gx = xq.tile([128, 1], F32, tag="gx")
                    nc.vector.tensor_reduce(out=gx[:, :], in_=xt[:, :],
                                            axis=AX.X, op=OP.max,
                                            apply_absolute_value=True)
                    nc.vector.tensor_scalar_max(out=gxall[:, m:m + 1],
                                                in0=gx[:, :], scalar1=1e-5)
                    rcp = xq.tile([128, 1], F32, tag="rcpx")
                    nc.vector.reciprocal(out=rcp[:, :],
                                         in_=gxall[:, m:m + 1])
                    sx = xq.tile([128, 1], F32, tag="sx")
                    nc.vector.tensor_scalar_mul(out=sx[:, :], in0=rcp[:, :],
                                                scalar1=127.0)
                    xs = xq.tile([128, D], F32, tag="x_sc")
                    nc.scalar.activation(out=xs[:, :], in_=xt[:, :],
                                         func=AF.Copy, scale=sx[:, :])
                    kx = xq.tile([128, D], BF16, tag="kx")
                    nc.vector.tensor_scalar(out=kx[:, :], in0=xs[:, :],
                                            scalar1=MAGIC, scalar2=MAGIC,
                                            op0=OP.add, op1=OP.subtract)
                    nc.sync.dma_start(out=kxT[:, :, m * 128:(m + 1) * 128],
                                      in_=kx[:, :], transpose=True)
            for k in range(KD):
                nc.sync.dma_start(out=kxb[k], in_=kxT[:, k, :])
            nc.sync.dma_start(
                out=gxb[0, :].rearrange("(m p) -> p m", p=128),
                in_=gxall[:, :])
        gp.collective_compute("AllGather", OP.bypass, replica_groups=RG,
                              ins=[kxb[:, :, :].opt()],
                              outs=[kxg[:, :, :, :].opt()])
        gp.collective_compute("AllGather", OP.bypass, replica_groups=RG,
                              ins=[gxb[:, :].opt()], outs=[gxg[:, :].opt()])

        # ================= phase W: weights =================
        Gv = gwT_d.ap().rearrange("(c p) h -> c p h", p=128)    # 16 x [128,HL]
        Vv = vwT_d.ap().rearrange("(c p) h -> c p h", p=128)
        Ov = owT_d.ap().rearrange("(c p) d -> c p d", p=128)    # 8 x [128,D]
        with tc.tile_pool(name="wW", bufs=1) as wW:
            WgT = wW.tile([128, KD, HL], BF16, tag="WgT")       # 4.2 MB
            WvT = wW.tile([128, KD, HL], BF16, tag="WvT")       # 4.2 MB
            with tc.tile_pool(name="wpa", bufs=3) as wpa:
                # pass A: |w| partial sums -> tiny AllReduce -> global gammas
                parts = wpa.tile([128, 4 * KD], F32, tag="parts", bufs=1)
                srcs = ([(Gv[c], c) for c in range(KD)]
                        + [(Vv[c], KD + c) for c in range(KD)]
                        + [(Ov[c][:, hf * HL:(hf + 1) * HL],
                            2 * KD + 2 * c + hf)
                           for c in range(KHL) for hf in range(2)])
                for src, col in srcs:
                    wt = wpa.tile([128, HL], F32, tag="ga_in")
                    nc.sync.dma_start(out=wt[:, :], in_=src)
                    scr = wpa.tile([128, HL], F32, tag="ga_scr")
                    nc.scalar.activation(out=scr[:, :], in_=wt[:, :],
                                         func=AF.Abs,
                                         accum_out=parts[:, col:col + 1])
                gsum = wpa.tile([128, 4], F32, tag="gsum", bufs=1)
                nc.vector.memset(gsum[:, :], 0.0)
                for j, sl in enumerate((slice(0, KD), slice(KD, 2 * KD),
                                        slice(2 * KD, 2 * KD + 2 * KHL))):
                    red = wpa.tile([128, 1], F32, tag="red")
                    nc.vector.tensor_reduce(out=red[:, :], in_=parts[:, sl],
                                            axis=AX.X, op=OP.add)
                    gp.partition_all_reduce(gsum[:, j:j + 1], red[:, :], 128,
                                            bass_isa.ReduceOp.add)
                nc.sync.dma_start(out=grb_i[0:1, :], in_=gsum[0:1, :])
                gp.collective_compute("AllReduce", OP.add, replica_groups=RG,
                                      ins=[grb_i[:, :].opt()],
                                      outs=[grb_o[:, :].opt()])
                g0 = wpa.tile([1, 4], F32, tag="g0")
                nc.sync.dma_start(out=g0[:, :], in_=grb_o[0:1, :])
                gbc = wpa.tile([128, 4], F32, tag="gbc")
                gp.partition_broadcast(gbc[:, :], g0[:, :])
                # gamma = clip(sum / (H*D), 1e-5); same count for all 3
                nc.vector.tensor_scalar(out=gam[:, :], in0=gbc[:, :],
                                        scalar1=1.0 / (H * D), scalar2=1e-5,
                                        op0=OP.mult, op1=OP.max)
                for j in range(3):
                    nc.vector.tensor_scalar_mul(
                        out=thr[:, 2 * j:2 * j + 1], in0=gam[:, j:j + 1],
                        scalar1=0.5)
                    nc.vector.tensor_scalar_mul(
                        out=thr[:, 2 * j + 1:2 * j + 2], in0=gam[:, j:j + 1],
                        scalar1=-0.5)
                # sel8 one-hot -> all partitions
                s0 = wpa.tile([1, N_CORES], F32, tag="s0")
                nc.sync.dma_start(out=s0[:, :], in_=sel_d.ap())
                gp.partition_broadcast(selb[:, :], s0[:, :])

            # pass B: ternarize into matmul layouts
            thr_g, nthr_g = thr[:, 0:1], thr[:, 1:2]
            thr_v, nthr_v = thr[:, 2:3], thr[:, 3:4]
            thr_o, nthr_o = thr[:, 4:5], thr[:, 5:6]
            with tc.tile_pool(name="wpb", bufs=3) as wpb:
                for c in range(KD):
                    wt = wpb.tile([128, HL], F32, tag="q_in")
                    nc.sync.dma_start(out=wt[:, :], in_=Gv[c])
                    sp = wpb.tile([128, HL], BF16, tag="q_sp")
                    nc.scalar.activation(out=sp[:, :], in_=wt[:, :],
                                         func=AF.Sign, bias=nthr_g)
                    sn = wpb.tile([128, HL], BF16, tag="q_sn")
                    nc.scalar.activation(out=sn[:, :], in_=wt[:, :],
                                         func=AF.Sign, bias=thr_g)
                    nc.vector.tensor_add(out=WgT[:, c, :], in0=sp[:, :],
                                         in1=sn[:, :])
                    wtv = wpb.tile([128, HL], F32, tag="q_in")
                    nc.sync.dma_start(out=wtv[:, :], in_=Vv[c])
                    mp = wpb.tile([128, HL], BF16, tag="q_sp")
                    nc.vector.tensor_scalar(out=mp[:, :], in0=wtv[:, :],
                                            scalar1=thr_v, scalar2=2.0,
                                            op0=OP.is_gt, op1=OP.mult)
                    mn = wpb.tile([128, HL], BF16, tag="q_sn")
                    nc.vector.tensor_scalar(out=mn[:, :], in0=wtv[:, :],
                                            scalar1=nthr_v, scalar2=2.0,
                                            op0=OP.is_lt, op1=OP.mult)
                    nc.vector.tensor_sub(out=WvT[:, c, :], in0=mp[:, :],
                                         in1=mn[:, :])
                for c in range(KHL):
                    for hf in range(2):
                        wt = wpb.tile([128, HL], F32, tag="q_in")
                        nc.sync.dma_start(
                            out=wt[:, :],
                            in_=Ov[c][:, hf * HL:(hf + 1) * HL])
                        sp = wpb.tile([128, HL], BF16, tag="q_sp")
                        nc.scalar.activation(out=sp[:, :], in_=wt[:, :],
                                             func=AF.Sign, bias=nthr_o)
                        sn = wpb.tile([128, HL], BF16, tag="q_sn")
                        nc.scalar.activation(out=sn[:, :], in_=wt[:, :],
                                             func=AF.Sign, bias=thr_o)
                        tq = wpb.tile([128, HL], BF16, tag="q_tq")
                        nc.vector.tensor_add(out=tq[:, :], in0=sp[:, :],
                                             in1=sn[:, :])
                        nc.sync.dma_start(
                            out=w2b[c * 128:(c + 1) * 128,
                                    hf * HL:(hf + 1) * HL],
                            in_=tq[:, :])
            gp.collective_compute("AllGather", OP.bypass, replica_groups=RG,
                                  ins=[w2b[:, :].opt()],
                                  outs=[w2g[:, :].opt()])

            # per-token mm1 eviction scales (all 64 global token chunks)
            with tc.tile_pool(name="scp", bufs=1) as scp:
                gxs = scp.tile([128, 64], F32, tag="gxs")
                nc.sync.dma_start(
                    out=gxs[:, :],
                    in_=gxg[:, :].rearrange("r (ml p) -> p (r ml)", p=128))
                nc.vector.tensor_scalar(out=s1a[:, :], in0=gxs[:, :],
                                        scalar1=gam[:, 0:1],
                                        scalar2=1.0 / 254.0,
                                        op0=OP.mult, op1=OP.mult)
                gx2 = scp.tile([128, 64], F32, tag="gx2")
                nc.vector.tensor_mul(out=gx2[:, :], in0=gxs[:, :],
                                     in1=gxs[:, :])
                t12 = scp.tile([128, 64], F32, tag="t12")
                nc.vector.tensor_scalar(out=t12[:, :], in0=gx2[:, :],
                                        scalar1=gam[:, 0:1],
                                        scalar2=gam[:, 1:2],
                                        op0=OP.mult, op1=OP.mult)
                nc.vector.tensor_scalar_mul(out=s12a[:, :], in0=t12[:, :],
                                            scalar1=1.0 / (254.0 * 254.0))

            # ============ phase M1: mm1 + requant + A2A ============
            with (
                tc.tile_pool(name="m1k", bufs=3) as m1k,
                tc.tile_pool(name="m1e", bufs=3) as m1e,
                tc.tile_pool(name="rq", bufs=2) as rq,
                tc.tile_pool(name="asm", bufs=3) as asm,
            ):
                for r in range(N_CORES):
                    hmall = m1e.tile([128, KHL], F32, tag="hmall", bufs=2,
                                     name=f"hmall{r}")
                    for ml in range(KHL):
                        m = r * KHL + ml
                        kxml = m1k.tile([128, KD, 128], BF16, tag="kxml")
                        nc.sync.dma_start(
                            out=kxml[:, :, :],
                            in_=kxg[r, :, :, ml * 128:(ml + 1) * 128]
                            .rearrange("k p t -> p k t"))
                        hm2 = m1e.tile([128, 2], F32, tag="hm2")
                        for n in range(2):
                            pg = psp.tile([128, 512], F32, tag="ps",
                                          name=f"pg{m}_{n}")
                            pv = psp.tile([128, 512], F32, tag="ps",
                                          name=f"pv{m}_{n}")
                            for k in range(KD):
                                lhsT = kxml[:, k, :]
                                nc.tensor.matmul(
                                    pg[:, :], lhsT=lhsT,
                                    rhs=WgT[:, k, n * 512:(n + 1) * 512],
                                    start=(k == 0), stop=(k == KD - 1))
                                nc.tensor.matmul(
                                    pv[:, :], lhsT=lhsT,
                                    rhs=WvT[:, k, n * 512:(n + 1) * 512],
                                    start=(k == 0), stop=(k == KD - 1))
                            A = m1e.tile([128, 512], F32, tag="Asb")
                            nc.scalar.activation(out=A[:, :], in_=pg[:, :],
                                                 func=AF.Sigmoid,
                                                 scale=s1a[:, m:m + 1])
                            t1 = m1e.tile([128, 512], F32, tag="t1sb")
                            nc.vector.scalar_tensor_tensor(
                                out=t1[:, :], in0=pg[:, :],
                                scalar=s12a[:, m:m + 1], in1=A[:, :],
                                op0=OP.mult, op1=OP.mult)
                            hs = m1e.tile([128, 512], F32, tag="hssb")
                            nc.vector.tensor_mul(out=hs[:, :], in0=pv[:, :],
                                                 in1=t1[:, :])
                            nc.vector.tensor_reduce(
                                out=hm2[:, n:n + 1], in_=hs[:, :], axis=AX.X,
                                op=OP.max, apply_absolute_value=True)
                            nc.sync.dma_start(
                                out=hsp[r][ml, :, n * 512:(n + 1) * 512],
                                in_=hs[:, :])
                        nc.vector.tensor_max(out=hmall[:, ml:ml + 1],
                                             in0=hm2[:, 0:1],
                                             in1=hm2[:, 1:2])
                    nc.sync.dma_start(
                        out=arh_i[r][0, :].rearrange("(ml p) -> p ml", p=128),
                        in_=hmall[:, :])
                    gp.collective_compute("AllReduce", OP.max,
                                          replica_groups=RG,
                                          ins=[arh_i[r][:, :].opt()],
                                          outs=[arh_o[r][:, :].opt()])
                    # requantize h block r with the global per-token max
                    ghr = m1e.tile([128, KHL], F32, tag="ghr", bufs=2,
                                   name=f"ghr{r}")
                    nc.sync.dma_start(
                        out=ghr[:, :],
                        in_=arh_o[r][0, :].rearrange("(ml p) -> p ml",
                                                     p=128))
                    gcl = m1e.tile([128, KHL], F32, tag="gcl", bufs=2,
                                   name=f"gcl{r}")
                    nc.vector.tensor_scalar_max(out=gcl[:, :], in0=ghr[:, :],
                                                scalar1=1e-5)
                    nc.vector.tensor_scalar(out=sofull[r][:, :],
                                            in0=gcl[:, :],
                                            scalar1=gam[:, 2:3],
                                            scalar2=1.0 / 254.0,
                                            op0=OP.mult, op1=OP.mult)
                    solm = m1e.tile([128, KHL], F32, tag="solm", bufs=2,
                                    name=f"solm{r}")
                    nc.vector.tensor_mul(out=solm[:, :], in0=sofull[r][:, :],
                                         in1=selb[:, :])
                    nc.vector.tensor_reduce(out=sosel[r][:, :],
                                            in_=solm[:, :], axis=AX.X,
                                            op=OP.add)
                    rcph = m1e.tile([128, KHL], F32, tag="rcph", bufs=2,
                                    name=f"rcph{r}")
                    nc.vector.reciprocal(out=rcph[:, :], in_=gcl[:, :])
                    shr = m1e.tile([128, KHL], F32, tag="shr", bufs=2,
                                   name=f"shr{r}")
                    nc.vector.tensor_scalar_mul(out=shr[:, :],
                                                in0=rcph[:, :],
                                                scalar1=127.0)
                    for ml in range(KHL):
                        hld = rq.tile([128, HL], F32, tag="hld")
                        nc.sync.dma_start(out=hld[:, :], in_=hsp[r][ml])
                        hmg = rq.tile([128, HL], F32, tag="hmg")
                        nc.scalar.activation(out=hmg[:, :], in_=hld[:, :],
                                             func=AF.Copy,
                                             scale=shr[:, ml:ml + 1],
                                             bias=MAGIC)
                        kh = rq.tile([128, HL], BF16, tag="kh")
                        nc.vector.tensor_scalar_sub(out=kh[:, :],
                                                    in0=hmg[:, :],
                                                    scalar1=MAGIC)
                        nc.sync.dma_start(out=a2i[r][ml], in_=kh[:, :])
                    gp.collective_compute("AllToAll", OP.bypass,
                                          replica_groups=RG,
                                          ins=[a2i[r][:, :, :].opt()],
                                          outs=[a2o[r][:, :, :].opt()])
                    # assemble mm2 lhsT k-half 1 (hidden sources j=0..3)
                    for j in range(N_CORES // 2):
                        khb = asm.tile([128, HL], BF16, tag="khb")
                        nc.sync.dma_start(out=khb[:, :], in_=a2o[r][j])
                        nc.sync.dma_start(
                            out=khT1[:, j * KHL:(j + 1) * KHL,
                                     r * 128:(r + 1) * 128],
                            in_=khb[:, :], transpose=True)

        # ================= phase M2: mm2 =================
        with (
            tc.tile_pool(name="m2p", bufs=1) as m2p,
            tc.tile_pool(name="m2a", bufs=3) as m2a,
            tc.tile_pool(name="m2w", bufs=4) as m2w,
            tc.tile_pool(name="m2o", bufs=4) as m2o,
        ):
            khT2 = m2p.tile([128, KH // 2, TL], BF16, tag="khT2")   # 8.4 MB
            for r in range(N_CORES):
                for j in range(N_CORES // 2, N_CORES):
                    khb = m2a.tile([128, HL], BF16, tag="khb2")
                    nc.sync.dma_start(out=khb[:, :], in_=a2o[r][j])
                    nc.sync.dma_start(
                        out=khT2[:, (j - 4) * KHL:(j - 3) * KHL,
                                 r * 128:(r + 1) * 128],
                        in_=khb[:, :], transpose=True)
            Wo = w2g[:, :].rearrange("(k p) d -> k p d", p=128)
            Outv = out_d.ap().rearrange("(r p) d -> r p d", p=128)
            for dcol in range(4):
                po = [psp.tile([128, 512], F32, tag="ps",
                               name=f"po{dcol}_{r}") for r in range(N_CORES)]
                for half, kht in ((0, khT1), (1, khT2)):
                    for kl in range(KH // 2):
                        k = half * (KH // 2) + kl
                        w2t = m2w.tile([128, 512], BF16, tag="w2t")
                        nc.sync.dma_start(
                            out=w2t[:, :],
                            in_=Wo[k][:, dcol * 512:(dcol + 1) * 512])
                        for r in range(N_CORES):
                            nc.tensor.matmul(
                                po[r][:, :],
                                lhsT=kht[:, kl, r * 128:(r + 1) * 128],
                                rhs=w2t[:, :],
                                start=(k == 0), stop=(k == KH - 1))
                for r in range(N_CORES):
                    ot = m2o.tile([128, 512], F32, tag="ot")
                    nc.scalar.activation(out=ot[:, :], in_=po[r][:, :],
                                         func=AF.Copy,
                                         scale=sosel[r][:, :])
                    nc.sync.dma_start(
                        out=Outv[r][:, dcol * 512:(dcol + 1) * 512],
                        in_=ot[:, :])


_NC_CACHE = {}


def _get_nc():
    if "nc" not in _NC_CACHE:
        _NC_CACHE["nc"] = _build()
    return _NC_CACHE["nc"]


def kernel(x, gate_w, gate_b, val_w, val_b, out_w, out_b, _trace=False):
    x = np.ascontiguousarray(np.asarray(x), dtype=np.float32)
    gate_w = np.asarray(gate_w, dtype=np.float32)
    val_w = np.asarray(val_w, dtype=np.float32)
    out_w = np.asarray(out_w, dtype=np.float32)
    gate_b = np.asarray(gate_b)
    val_b = np.asarray(val_b)
    out_b = np.asarray(out_b)
    assert not np.any(gate_b) and not np.any(val_b), (
        "device kernel folds silu(y+b) with b=0; nonzero gate/val bias "
        "not supported")

    orig_shape = x.shape
    xf = x.reshape(-1, x.shape[-1])
    assert xf.shape == (T, D) and gate_w.shape == (H, D)
    assert val_w.shape == (H, D) and out_w.shape == (D, H)

    nc = _get_nc()
    in_maps = []
    for i in range(N_CORES):
        sel = np.zeros((1, N_CORES), np.float32)
        sel[0, i] = 1.0
        in_maps.append({
            "x": xf[i * TL:(i + 1) * TL],
            "gwT": np.ascontiguousarray(gate_w[i * HL:(i + 1) * HL, :].T),
            "vwT": np.ascontiguousarray(val_w[i * HL:(i + 1) * HL, :].T),
            "owT": np.ascontiguousarray(out_w[:, i * HL:(i + 1) * HL].T),
            "sel8": sel,
        })
    res = run_bass_kernel_spmd(nc, in_maps, core_ids=list(range(N_CORES)),
                               trace=_trace)
    # core i owns tokens r*1024 + i*128 + [0,128) for r in 0..7
    out = np.empty((T, D), np.float32)
    ov = out.reshape(N_CORES, N_CORES, 128, D)       # [r, i, p, d]
    for i in range(N_CORES):
        ov[:, i] = res.results[i]["out"].reshape(N_CORES, 128, D)
    out = out + out_b[None, :].astype(np.float32)
    kernel._last_results = res
    return out.reshape(orig_shape)
